# revision 24
# baseline (speedup 1.0000x reference)
import sys
import types
import random
sys.path.insert(0, '/opt/trn_rl_repo')
import numpy as np
import ml_dtypes

# Ensure the NTFF profile hook registry exists (some images lack
# antenv.axon_hooks; bass_utils imports it when BASS_TRACE=1).
try:
    from antenv import axon_hooks as _ah  # noqa: F401
except ImportError:
    import antenv as _antenv
    _m = types.ModuleType('antenv.axon_hooks')
    _m._hook = None

    def _set_hook(h):
        _m._hook = h

    def _get_hook():
        if _m._hook is None:
            try:
                from trn_agent_boot.trn_boot import _ntff_profile_via_ctypes
                _m._hook = _ntff_profile_via_ctypes('/opt/axon/libaxon_pjrt.so')
            except Exception:
                return None
        return _m._hook

    _m.set_axon_ntff_profile_hook = _set_hook
    _m.get_axon_ntff_profile_hook = _get_hook
    sys.modules['antenv.axon_hooks'] = _m
    _antenv.axon_hooks = _m

import concourse.bass as bass
import concourse.bacc as bacc
import concourse.mybir as mybir
from concourse import tile
from concourse.bass_utils import run_bass_kernel_spmd

BF16 = ml_dtypes.bfloat16
N, C, D, H, W = 8, 32, 64, 64, 64
NB = 256
CD = CH = CW = 16
NCORES = 8
BPC = NB // NCORES  # boxes (slots) per core

last_exec_ns = None


def _axis_tables(lo, hi, L):
    # follows reference._coords/_lerp_idx in float32
    i = np.arange(CD, dtype=np.float32)
    step = (hi - lo) * (L - 1) / (CD - 1)
    coord = lo * (L - 1) + i * step
    coord = np.clip(coord, 0.0, L - 1)
    i0 = np.floor(coord).astype(np.int64)
    frac = (coord - i0).astype(np.float32)
    # remap i0 == L-1 so that i1 = i0+1 always stays in range
    sel = i0 == L - 1
    i0[sel] = L - 2
    frac[sel] = 1.0
    return i0, frac


def _pair_weights(iabs, i0, frac):
    # weight of absolute index iabs for each of the 16 outputs; indices
    # outside [0, L-1] simply match nothing -> zero weight
    a = (iabs[..., None] == i0) * (1.0 - frac)
    b = (iabs[..., None] == i0 + 1) * frac
    return (a + b).astype(np.float32)


def _slot_shape(bs):
    # slot-uniform shape for a group of boxes
    ybar = max(g['ys'] for g in bs)
    zmax = max(g['zneed'] for g in bs)
    m = max(1, min(128 // ybar, zmax))
    J = -(-zmax // m)
    wbar = max(g['wbar'] for g in bs)
    return ybar, m, J, wbar


def _slot_cost(bs):
    ybar, m, J, wbar = _slot_shape(bs)
    nh = wbar // 16
    rows = (J * nh * 4 + nh * 4) * 256
    dmab = 128 * J * 4 * wbar * 8 * 2
    return rows * 0.4167 + dmab / 358e3 * 1000 * 0.2


def kernel(image, boxes, box_ind):
    global last_exec_ns
    image = np.asarray(image, dtype=np.float32)
    boxes = np.asarray(boxes, dtype=np.float32)
    box_ind = np.asarray(box_ind)

    # [n, Q, c8, z, y, x] view for host-side slab extraction
    imgq = image.reshape(N, 4, 8, D, H, W)

    # ---- per-box geometry ----
    geos = []
    for b in range(NB):
        x1, y1, z1, x2, y2, z2 = boxes[b]
        z0, fz = _axis_tables(z1, z2, D)
        y0, fy = _axis_tables(y1, y2, H)
        x0, fx = _axis_tables(x1, x2, W)
        n = int(box_ind[b])
        wneed = int(x0.max() + 2 - x0.min())
        wbar = min(64, ((wneed + 15) // 16) * 16)
        ys = int(y0.max() + 2 - y0.min())
        zneed = int(z0.max() + 2 - z0.min())
        geos.append(dict(n=n, z0=z0, fz=fz, y0=y0, fy=fy, x0=x0, fx=fx,
                         wbar=wbar, ys=ys, zneed=zneed,
                         zlo=int(z0.min()), ylo=int(y0.min()), xs=int(x0.min())))

    # ---- grouping: lexicographic seed + multi-restart local-swap optimizer ----
    base = sorted(range(NB), key=lambda b: (-geos[b]['ys'], -geos[b]['wbar'], -geos[b]['zneed']))
    best_order, best_cost = None, None
    for seed in (12345, 777, 31337):
        order = list(base)
        rng = random.Random(seed)
        costs = [_slot_cost([geos[b] for b in order[s * 8:(s + 1) * 8]]) for s in range(BPC)]
        for _ in range(300000):
            i, j = rng.randrange(NB), rng.randrange(NB)
            si, sj = i // 8, j // 8
            if si == sj:
                continue
            oi, oj = order[i], order[j]
            a = [geos[order[k]] if k != i else geos[oj] for k in range(si * 8, si * 8 + 8)]
            bsl = [geos[order[k]] if k != j else geos[oi] for k in range(sj * 8, sj * 8 + 8)]
            na, nb_ = _slot_cost(a), _slot_cost(bsl)
            if na + nb_ < costs[si] + costs[sj] - 1e-9:
                order[i], order[j] = oj, oi
                costs[si], costs[sj] = na, nb_
        tot = sum(costs)
        if best_cost is None or tot < best_cost:
            best_order, best_cost = list(order), tot
    order = best_order
    slot_boxes = [[order[s * NCORES + c] for c in range(NCORES)] for s in range(BPC)]

    slots = []
    for s in range(BPC):
        bs = [geos[b] for b in slot_boxes[s]]
        ybar, m, J, wbar = _slot_shape(bs)
        P = 128  # pad partitions: even DMA-engine spread; extra B rows are zero
        slots.append(dict(ybar=ybar, m=m, J=J, wbar=wbar, P=P, sb=slot_boxes[s],
                          big=(P * J * 4 * wbar * 8 * 2) > (2 << 20)))

    # interleave big and small slots so big slab loads overlap small-slot
    # compute: big slots spread evenly through the program order
    bigs = [sl for sl in slots if sl['big']]
    smalls = [sl for sl in slots if not sl['big']]
    inter = []
    nb_, ns_ = len(bigs), len(smalls)
    bi = si = 0
    for k in range(BPC):
        # strict > delays each big slot past its even-spread point, so the
        # program opens with small slots and the PE ramps while big slabs load
        if bi < nb_ and (si >= ns_ or k * nb_ > bi * (ns_ + nb_)):
            inter.append(bigs[bi]); bi += 1
        else:
            inter.append(smalls[si]); si += 1
    slots = inter
    slot_boxes = [sl['sb'] for sl in slots]

    # ---- per-core weight tables + host-gathered slabs ----
    bts = [[] for _ in range(NCORES)]
    wxs = [[] for _ in range(NCORES)]
    slabs = [[] for _ in range(NCORES)]
    bt_offs, wx_offs, slab_offs = [], [], []
    ob, ow, osl = 0, 0, 0
    for s, sl in enumerate(slots):
        J, m, ybar, wbar, P = sl['J'], sl['m'], sl['ybar'], sl['wbar'], sl['P']
        bt_offs.append(ob); wx_offs.append(ow); slab_offs.append(osl)
        ob += J * 256; ow += (wbar // 16) * 128
        osl += P * J * 4 * wbar * 8
        Pr = m * ybar  # real (un-padded) slab partition rows
        p_arr = np.arange(Pr)
        zr = p_arr // ybar
        yr = p_arr % ybar
        for c in range(NCORES):
            g = geos[slot_boxes[s][c]]
            zlo, ylo, xs = g['zlo'], g['ylo'], g['xs']
            # B [Pr, J, 256] -> padded to 128 rows
            zabs = zlo + np.arange(J)[:, None] * m + zr[None, :]     # [J,Pr] (raw)
            wz = _pair_weights(zabs, g['z0'], g['fz'])               # [J,Pr,16]
            wyv = _pair_weights(ylo + yr, g['y0'], g['fy'])          # [Pr,16]
            B = np.einsum('jpz,py->pjzy', wz, wyv).reshape(Pr, J * 256)
            if Pr < 128:
                B = np.concatenate([B, np.zeros((128 - Pr, J * 256), np.float32)])
            bts[c].append(B.astype(BF16))
            # Wx [128, (wbar//16)*128]: blk h: [r*8+c8, c8p*16+xo]
            xabs = xs + np.arange(wbar)                              # [wbar] (raw)
            wxv = _pair_weights(xabs, g['x0'], g['fx'])              # [wbar,16]
            nh = wbar // 16
            blk = np.zeros((nh, 16, 8, 8, 16), dtype=np.float32)
            for c8 in range(8):
                blk[:, :, c8, c8, :] = wxv.reshape(nh, 16, 16)
            wxs[c].append(blk.reshape(nh, 128, 128).transpose(1, 0, 2).reshape(128, nh * 128).astype(BF16))
            # slab gather with clipped indices (weights are zero out of range)
            zi = np.clip(zlo + (np.arange(J)[:, None] * m + np.arange(m)[None, :]).ravel(), 0, D - 1)
            yi = np.clip(ylo + np.arange(ybar), 0, H - 1)
            xi = np.clip(xs + np.arange(wbar), 0, W - 1)
            sub = imgq[g['n']][:, :, zi][:, :, :, yi][:, :, :, :, xi]  # [4,8,J*m,ybar,wbar]
            sub = sub.reshape(4, 8, J, m, ybar, wbar)
            if sl['big']:
                # [Q, m, ybar, J, wbar, c8] -> [4, Pr, J*wbar*8] -> pad rows to 128
                arr = sub.transpose(0, 3, 4, 2, 5, 1).reshape(4, Pr, -1)
                if Pr < 128:
                    arr = np.concatenate(
                        [arr, np.zeros((4, 128 - Pr, arr.shape[2]), arr.dtype)], axis=1)
            else:
                # [m, ybar, J, Q, wbar, c8] -> [Pr, J*4*wbar*8] -> pad rows to 128
                arr = sub.transpose(3, 4, 2, 0, 5, 1).reshape(Pr, -1)
                if Pr < 128:
                    arr = np.concatenate(
                        [arr, np.zeros((128 - Pr, arr.shape[1]), arr.dtype)], axis=0)
            slabs[c].append(np.ascontiguousarray(arr).astype(BF16).ravel())
    bt_np = [np.concatenate(bts[c], axis=1) for c in range(NCORES)]
    wx_np = [np.concatenate(wxs[c], axis=1) for c in range(NCORES)]
    slab_np = [np.concatenate(slabs[c]) for c in range(NCORES)]
    TOTB, TOTW, TOTS = bt_np[0].shape[1], wx_np[0].shape[1], slab_np[0].size

    # ---- build device program (identical across cores; data differs) ----
    nc = bacc.Bacc("TRN2", target_bir_lowering=False, debug=False)
    slab_t = nc.dram_tensor("slab", [TOTS], mybir.dt.bfloat16, kind="ExternalInput")
    bt_t = nc.dram_tensor("bt", [128, TOTB], mybir.dt.bfloat16, kind="ExternalInput")
    wx_t = nc.dram_tensor("wx", [128, TOTW], mybir.dt.bfloat16, kind="ExternalInput")
    out_t = nc.dram_tensor("out", [BPC, 128, 1024], mybir.dt.bfloat16, kind="ExternalOutput")

    with tile.TileContext(nc) as tc:
        with tc.tile_pool(name="gf", bufs=3) as gfp, \
             tc.tile_pool(name="gq", bufs=3) as gqp, \
             tc.tile_pool(name="wt", bufs=2) as wtp, \
             tc.tile_pool(name="x1", bufs=3) as x1p, \
             tc.tile_pool(name="oo", bufs=2) as oop, \
             tc.tile_pool(name="ps", bufs=4, space="PSUM") as psp:
            for s, sl in enumerate(slots):
                J, m, ybar, wbar, P = sl['J'], sl['m'], sl['ybar'], sl['wbar'], sl['P']
                nh = wbar // 16
                slab_eng = nc.sync if (s % 2 == 0) else nc.scalar
                aux_eng = nc.scalar if (s % 2 == 0) else nc.sync
                btile = wtp.tile([128, J * 256], mybir.dt.bfloat16, tag="bt")
                aux_eng.dma_start(out=btile[:], in_=bt_t[:, bt_offs[s]:bt_offs[s] + J * 256])
                wtile = wtp.tile([128, nh * 128], mybir.dt.bfloat16, tag="wx")
                aux_eng.dma_start(out=wtile[:], in_=wx_t[:, wx_offs[s]:wx_offs[s] + nh * 128])
                O = oop.tile([128, 1024], mybir.dt.bfloat16)
                qgroups = [[0], [1], [2], [3]] if sl['big'] else [[0, 1, 2, 3]]
                rowlen = J * wbar * 8 * (1 if sl['big'] else 4)
                for Qs in qgroups:
                    G = (gqp if sl['big'] else gfp).tile(
                        [P, J, len(Qs), wbar * 8], mybir.dt.bfloat16,
                        tag="gq" if sl['big'] else "gf")
                    off = slab_offs[s] + (Qs[0] * P * rowlen if sl['big'] else 0)
                    if sl['big'] and J >= 2:
                        # split each chunk across both HWDGE rings concurrently
                        J2 = J // 2
                        h1 = J2 * wbar * 8
                        nc.sync.dma_start(
                            out=G[:, 0:J2, :, :],
                            in_=bass.AP(slab_t, off, [[rowlen, P], [1, h1]]))
                        nc.scalar.dma_start(
                            out=G[:, J2:J, :, :],
                            in_=bass.AP(slab_t, off + h1, [[rowlen, P], [1, rowlen - h1]]))
                    else:
                        geng = (nc.sync if Qs[0] % 2 == 0 else nc.scalar) if sl['big'] else slab_eng
                        geng.dma_start(
                            out=G[:],
                            in_=bass.AP(slab_t, off, [[rowlen, P], [1, rowlen]]))
                    for qi, Q in enumerate(Qs):
                        X1 = x1p.tile([128, nh, 256], mybir.dt.bfloat16)
                        for h in range(nh):
                            psA = psp.tile([128, 256], mybir.dt.float32)
                            for j in range(J):
                                nc.tensor.matmul(
                                    out=psA[:],
                                    lhsT=G[:, j, qi, 128 * h:128 * (h + 1)],
                                    rhs=btile[:P, 256 * j:256 * (j + 1)],
                                    start=(j == 0), stop=(j == J - 1))
                            nc.vector.tensor_copy(X1[:, h, :], psA[:])
                        psB = psp.tile([128, 256], mybir.dt.float32)
                        for h in range(nh):
                            nc.tensor.matmul(
                                out=psB[:], lhsT=wtile[:, 128 * h:128 * (h + 1)],
                                rhs=X1[:, h, :], start=(h == 0), stop=(h == nh - 1))
                        nc.vector.tensor_copy(O[:, 256 * Q:256 * (Q + 1)], psB[:])
                aux_eng.dma_start(out=out_t[s], in_=O[:])
    nc.compile()

    in_maps = [{"slab": slab_np[c], "bt": bt_np[c], "wx": wx_np[c]} for c in range(NCORES)]
    res = run_bass_kernel_spmd(nc, in_maps, list(range(NCORES)), trace=False)
    globals()['last_res'] = res
    last_exec_ns = res.exec_time_ns

    # ---- host: reassemble ----
    out = np.zeros((NB, C, CD, CH, CW), dtype=np.float32)
    for s in range(BPC):
        for c in range(NCORES):
            b = slot_boxes[s][c]
            o = res.results[c]["out"][s].astype(np.float32)  # [128, 1024]
            # p = c8*16+xo ; free = Q*256 + zo*16 + yo
            o = o.reshape(8, 16, 4, 16, 16)          # [c8, xo, Q, zo, yo]
            out[b] = o.transpose(2, 0, 3, 4, 1).reshape(C, CD, CH, CW)
    return out
